# revision 14
# baseline (speedup 1.0000x reference)
"""Trainium2 Bass kernel for nn_JCAF: 3-branch cross-attention fusion module.

Strategy (8 NeuronCores, pure data-parallel over batch B=64 -> 8 batches/core):
  - All matmuls in bf16 (fp32 PSUM accumulation), elementwise in fp32.
  - Reassociated attention chain:  att^T = G_src^T (W_aff @ feats) / 16
    computed as Y = W_aff @ feats first ([L,L]@[L,D]), saving ~45% FLOPs vs
    the reference order.
  - Global norms n1=|f1|, n2=|f2| via the Gram trick: each core computes
    S = X^T X on-device (bf16 matmuls), n^2 = <S, W W^T> + host colsum bias
    terms; partial n^2 scalars are AllReduced across the 8 cores on-device.
  - z/G computed in transposed layout [D, L] so AvgPool+L2-normalize become
    free-dim ops; G transposed back natural with 128x128 PE transposes.
  - 4-batch matmul grouping (free dim 512) for the big matmuls.
"""

import sys

sys.path.insert(0, "/opt/trn_rl_repo")

import numpy as np
import ml_dtypes
from contextlib import ExitStack

B, L, D, K = 64, 1024, 128, 256
NCORES = 8
BLOC = B // NCORES  # 8
NG = 2              # batch groups per core
GB = 4              # batches per group
LC = L // 128       # 8 l-chunks

bf16 = ml_dtypes.bfloat16

_cache = {}


def _build_nc():
    import concourse.bacc as bacc
    import concourse.tile as tile
    import concourse.mybir as mybir
    from concourse.masks import make_identity

    mdt = mybir.dt
    AF = mybir.ActivationFunctionType
    ALU = mybir.AluOpType

    nc = bacc.Bacc("TRN2", target_bir_lowering=False, debug=False,
                   enable_asserts=False, num_devices=NCORES)

    # ---- DRAM I/O ----
    x4_d = nc.dram_tensor("x4", [3, NG, LC, 128, GB * 128], mdt.bfloat16,
                          kind="ExternalInput").ap()
    xT_d = nc.dram_tensor("xT", [2, BLOC, 128, L], mdt.bfloat16,
                          kind="ExternalInput").ap()
    wt_d = nc.dram_tensor("wt", [3, LC, 128, L], mdt.bfloat16,
                          kind="ExternalInput").ap()
    wlin_d = nc.dram_tensor("wlin", [3, LC, 128, K], mdt.bfloat16,
                            kind="ExternalInput").ap()
    wc_d = nc.dram_tensor("wc", [3, 2, 128, K], mdt.bfloat16,
                          kind="ExternalInput").ap()
    wh_d = nc.dram_tensor("wh", [3, 2, 128, L], mdt.bfloat16,
                          kind="ExternalInput").ap()
    wp_d = nc.dram_tensor("wp", [2, 128, 128], mdt.bfloat16,
                          kind="ExternalInput").ap()
    cbv_d = nc.dram_tensor("cbv", [128, 128], mdt.float32,
                           kind="ExternalInput").ap()
    out_d = [nc.dram_tensor(f"out{r}", [BLOC, L, D], mdt.float32,
                            kind="ExternalOutput").ap() for r in range(3)]

    with tile.TileContext(nc) as tc, ExitStack() as ctx:
        wpool = ctx.enter_context(tc.tile_pool(name="wpool", bufs=1))
        xpool = ctx.enter_context(tc.tile_pool(name="xpool", bufs=1))
        xtpool = ctx.enter_context(tc.tile_pool(name="xtpool", bufs=4))
        g4pool = ctx.enter_context(tc.tile_pool(name="g4pool", bufs=1))
        y4pool = ctx.enter_context(tc.tile_pool(name="y4pool", bufs=2))
        sbw = ctx.enter_context(tc.tile_pool(name="sbw", bufs=2))
        ps_big = ctx.enter_context(tc.tile_pool(name="ps_big", bufs=4, space="PSUM"))
        ps_sm = ctx.enter_context(tc.tile_pool(name="ps_sm", bufs=3, space="PSUM"))
        ps_d = ctx.enter_context(tc.tile_pool(name="ps_d", bufs=1, space="PSUM"))

        # ---- weights / constants ----
        wt_s = [[wpool.tile([128, L], mdt.bfloat16, name=f"wt{r}_{lc}")
                 for lc in range(LC)] for r in range(3)]
        wlin_s = [[wpool.tile([128, K], mdt.bfloat16, name=f"wlin{r}_{lc}")
                   for lc in range(LC)] for r in range(3)]
        wc_s = [[wpool.tile([128, K], mdt.bfloat16, name=f"wc{r}_{cc}")
                 for cc in range(2)] for r in range(3)]
        wh_s = [[wpool.tile([128, L], mdt.bfloat16, name=f"wh{r}_{kc}")
                 for kc in range(2)] for r in range(3)]
        for r in range(3):
            for lc in range(LC):
                nc.sync.dma_start(wt_s[r][lc][:], wt_d[r, lc])
                nc.sync.dma_start(wlin_s[r][lc][:], wlin_d[r, lc])
            for cc in range(2):
                nc.sync.dma_start(wc_s[r][cc][:], wc_d[r, cc])
                nc.sync.dma_start(wh_s[r][kc := cc][:], wh_d[r, kc])
        wp_s = [wpool.tile([128, 128], mdt.bfloat16, name=f"wp{t}") for t in range(2)]
        for t in range(2):
            nc.sync.dma_start(wp_s[t][:], wp_d[t])
        cbv_s = wpool.tile([128, 128], mdt.float32, name="cbv")
        nc.sync.dma_start(cbv_s[:], cbv_d)
        onesb = wpool.tile([128, 128], mdt.bfloat16, name="onesb")
        nc.vector.memset(onesb[:], 1.0)

        # ---- feature tiles (natural layout, 4-batch grouped) ----
        x4_s = [[[xpool.tile([128, GB * 128], mdt.bfloat16, name=f"x4_{t}_{g}_{lc}")
                  for lc in range(LC)] for g in range(NG)] for t in range(3)]
        for t in range(3):
            for g in range(NG):
                for lc in range(LC):
                    nc.sync.dma_start(x4_s[t][g][lc][:], x4_d[t, g, lc])

        # ---- stage 2: biamlp -> G in natural layout (no transposes) ----
        # z_chunk[l,d] = txt @ (w1*Wp_i) + aud @ (w2*Wp_q) + cbv   (one PSUM group)
        # denom^2 via ones-matmul (result pre-broadcast across partitions)
        g4_s = [[g4pool.tile([128, GB * 128], mdt.bfloat16, name=f"g4_{g}_{lc}")
                 for lc in range(LC)] for g in range(NG)]
        for b in range(BLOC):
            g, bb = divmod(b, GB)
            bsl = slice(bb * 128, (bb + 1) * 128)
            xt_t = xtpool.tile([128, L], mdt.bfloat16, tag="xt")
            au_t = xtpool.tile([128, L], mdt.bfloat16, tag="au")
            nc.sync.dma_start(xt_t[:], xT_d[0, b])
            nc.sync.dma_start(au_t[:], xT_d[1, b])
            dsq = ps_d.tile([128, 128], mdt.float32, tag="dsq")
            zc_l = []
            for lc in range(LC):
                lsl = slice(lc * 128, (lc + 1) * 128)
                zp = ps_sm.tile([128, 128], mdt.float32, tag="small")
                nc.tensor.matmul(zp[:], lhsT=xt_t[:, lsl], rhs=wp_s[0][:],
                                 start=True, stop=False)
                nc.tensor.matmul(zp[:], lhsT=au_t[:, lsl], rhs=wp_s[1][:],
                                 start=False, stop=True)
                zc = sbw.tile([128, 128], mdt.float32, tag=f"zc{lc}")
                nc.vector.tensor_tensor(zc[:], zp[:], cbv_s[:], ALU.add)
                z2 = sbw.tile([128, 128], mdt.bfloat16, tag="z2")
                nc.scalar.activation(z2[:], zc[:], AF.Square)
                nc.tensor.matmul(dsq[:], lhsT=onesb[:], rhs=z2[:],
                                 start=(lc == 0), stop=(lc == LC - 1))
                zc_l.append(zc)
            rden = sbw.tile([128, 128], mdt.float32, tag="rden")
            nc.scalar.activation(rden[:], dsq[:], AF.Sqrt)
            nc.vector.tensor_scalar_max(rden[:], rden[:], 1e-12)
            nc.vector.reciprocal(rden[:], rden[:])
            for lc in range(LC):
                nc.vector.tensor_tensor(g4_s[g][lc][:, bsl], zc_l[lc][:],
                                        rden[:], ALU.mult)

        # ---- stage 3: branches ----
        # r=0: txt (gfirst=txt), r=1: aud, r=2: vis (gfirst=aud, bug preserved)
        for g in range(NG):
            for r in range(3):
                gf = 0 if r == 0 else 1
                # Y4: [l''c][128, 512] = W_aff @ feats for 4 batches
                y4 = []
                for mc in range(LC):
                    yp = ps_big.tile([128, 512], mdt.float32, tag="big")
                    for lc in range(LC):
                        nc.tensor.matmul(
                            yp[:], lhsT=wt_s[r][lc][:, mc * 128:(mc + 1) * 128],
                            rhs=x4_s[r][g][lc][:], start=(lc == 0),
                            stop=(lc == LC - 1))
                    yt = y4pool.tile([128, 512], mdt.bfloat16, tag=f"y4_{mc}")
                    nc.scalar.copy(yt[:], yp[:])
                    y4.append(yt)
                # attT + tanh -> ct4 [cc][128, 512] bf16 (4 batches side by side)
                ct4 = [sbw.tile([128, 512], mdt.bfloat16, tag=f"ct4_{cc}",
                                name=f"ct4_{g}_{r}_{cc}")
                       for cc in range(2)]
                for bb in range(GB):
                    bsl = slice(bb * 128, (bb + 1) * 128)
                    for cc in range(2):
                        ap = ps_sm.tile([128, 128], mdt.float32, tag="small")
                        for mc in range(LC):
                            lhs = (x4_s[gf][g][mc][:, bsl] if cc == 0
                                   else g4_s[g][mc][:, bsl])
                            nc.tensor.matmul(ap[:], lhsT=lhs,
                                             rhs=y4[mc][:, bsl],
                                             start=(mc == 0),
                                             stop=(mc == LC - 1))
                        nc.scalar.activation(ct4[cc][:, bsl], ap[:], AF.Tanh,
                                             scale=1.0 / 16.0)
                # HT4: [kc][128, 512] = relu(W_c^T CT + W_lin^T feats)
                ht4 = []
                for kc in range(2):
                    hp = ps_big.tile([128, 512], mdt.float32, tag="big")
                    for lc in range(LC):
                        nc.tensor.matmul(
                            hp[:], lhsT=wlin_s[r][lc][:, kc * 128:(kc + 1) * 128],
                            rhs=x4_s[r][g][lc][:], start=(lc == 0), stop=False)
                    for cc in range(2):
                        nc.tensor.matmul(
                            hp[:], lhsT=wc_s[r][cc][:, kc * 128:(kc + 1) * 128],
                            rhs=ct4[cc][:], start=False, stop=(cc == 1))
                    ht = sbw.tile([128, 512], mdt.bfloat16, tag=f"ht4_{kc}")
                    nc.scalar.activation(ht[:], hp[:], AF.Relu)
                    ht4.append(ht)
                # out4: [lc][128, 512] = W_h^T HT + feats -> DRAM
                for lc in range(LC):
                    op = ps_big.tile([128, 512], mdt.float32, tag="big")
                    for kc in range(2):
                        nc.tensor.matmul(
                            op[:], lhsT=wh_s[r][kc][:, lc * 128:(lc + 1) * 128],
                            rhs=ht4[kc][:], start=(kc == 0), stop=(kc == 1))
                    res = sbw.tile([128, 512], mdt.float32, tag="res")
                    nc.vector.tensor_tensor(res[:], op[:], x4_s[r][g][lc][:],
                                            ALU.add)
                    dst = out_d[r][g * GB:(g + 1) * GB,
                                   lc * 128:(lc + 1) * 128, :]
                    nc.sync.dma_start(
                        dst.rearrange("b l d -> l b d"),
                        res[:].rearrange("p (b d) -> p b d", b=GB))

    nc.compile()
    return nc


def _prep_core(inputs, c):
    """Host-side prep of one core's input map."""
    f32 = np.float32
    sl = slice(c * BLOC, (c + 1) * BLOC)
    txt, aud, vis = (inputs['f1_norm'][sl], inputs['f2_norm'][sl],
                     inputs['f3_norm'][sl])
    x4 = np.empty((3, NG, LC, 128, GB * 128), bf16)
    for t, arr in enumerate((txt, aud, vis)):
        x4[t] = (arr.astype(bf16).reshape(NG, GB, LC, 128, 128)
                 .transpose(0, 2, 3, 1, 4).reshape(NG, LC, 128, GB * 128))
    xT = np.empty((2, BLOC, 128, L), bf16)
    for t, arr in enumerate((txt, aud)):
        xT[t] = np.ascontiguousarray(arr.astype(bf16).transpose(0, 2, 1))
    return {"x4": x4, "xT": xT}


def _prep_shared(inputs):
    f32 = np.float32
    affs = ('Wl_aff', 'Wa_aff', 'Wv_aff')
    wlins = ('W_t', 'W_a', 'W_v')
    wcs = ('W_ct', 'W_ca', 'W_cv')
    whs = ('W_ht', 'W_ha', 'W_hv')
    wt = np.empty((3, LC, 128, L), bf16)
    wlin = np.empty((3, LC, 128, K), bf16)
    wc = np.empty((3, 2, 128, K), bf16)
    wh = np.empty((3, 2, 128, L), bf16)
    for r in range(3):
        wt[r] = np.ascontiguousarray(inputs[affs[r]].T).astype(bf16) \
            .reshape(LC, 128, L)
        wlin[r] = inputs[wlins[r]].astype(bf16).reshape(LC, 128, K)
        wc[r] = inputs[wcs[r]].astype(bf16).reshape(2, 128, K)
        wh[r] = inputs[whs[r]].astype(bf16).reshape(2, 128, L)
    Wi, bi, Wq, bq = (inputs['Wi'], inputs['bi'], inputs['Wq'], inputs['bq'])
    # global norms on host (cheap: 2x [65536,128]@[128,256])
    f1 = inputs['f1_norm'].reshape(-1, D) @ Wi + bi
    f2 = inputs['f2_norm'].reshape(-1, D) @ Wq + bq
    n1 = float(np.sqrt((f1.astype(np.float64) ** 2).sum()))
    n2 = float(np.sqrt((f2.astype(np.float64) ** 2).sum()))
    w1, w2 = n1 / (n1 + n2), n2 / (n1 + n2)
    wp = np.stack([(w1 * (Wi[:, 0::2] + Wi[:, 1::2])).astype(bf16),
                   (w2 * (Wq[:, 0::2] + Wq[:, 1::2])).astype(bf16)])
    cbv_row = (w1 * (bi[0::2] + bi[1::2]) + w2 * (bq[0::2] + bq[1::2]))
    cbv = np.ascontiguousarray(
        np.broadcast_to(cbv_row.astype(f32), (128, 128)))
    return {"wt": wt, "wlin": wlin, "wc": wc, "wh": wh, "wp": wp, "cbv": cbv}


def kernel(**inputs):
    from concourse import bass_utils

    if "nc" not in _cache:
        _cache["nc"] = _build_nc()
    nc = _cache["nc"]

    shared = _prep_shared(inputs)
    in_maps = []
    for c in range(NCORES):
        m = dict(shared)
        m.update(_prep_core(inputs, c))
        in_maps.append(m)

    res = bass_utils.run_bass_kernel_spmd(nc, in_maps,
                                          core_ids=list(range(NCORES)))
    outs = []
    for r in range(3):
        outs.append(np.concatenate(
            [res.results[c][f"out{r}"] for c in range(NCORES)], axis=0))
    return tuple(outs)


if __name__ == "__main__":
    d = np.load("/root/problem/work/inputs.npz")
    e = np.load("/root/problem/work/expected.npz")
    outs = kernel(**{k: d[k] for k in d.files})
    for r, name in enumerate(("txt", "aud", "vis")):
        exp = e[name]
        rel = np.abs(outs[r] - exp).max() / np.abs(exp).max()
        print(name, "relmax:", rel)
